# revision 1
# baseline (speedup 1.0000x reference)
"""Trainium2 Bass kernel for CrossDMHAttention (B=131072 single-query cross
attention, T=24, H=4 heads, head_dim 8, + LayerNorm + residual GELU MLP),
data-parallel over 8 NeuronCores.

Transposed bf16 dataflow: features on SBUF partitions, batch rows along the
free dimension in 512-row tiles. The host pre-transposes and casts inputs to
bf16 (kv re-tiled so each partition reads one contiguous 6KB strip per tile;
the 13->128 query projection is precomputed on host), so every DMA is
contiguous. The output is written transposed [32, rows] fp32 and transposed
back on the host.

Per tile, pass A (attention + layernorm):
  PE     q/k/v projections via block-diagonal weights (4 tokens per matmul);
         score head-dim reduction, softmax denominator, -ln(denom)
         subtraction, ctx token-reduction fused with Wo and -mean, all as
         matmuls with constant selection weights accumulated in PSUM.
  ACT    exp/ln softmax pieces (kept in the one act table that has both),
         Square; rstd = exp(-0.5*ln(var+eps)) avoids the sqrt table.
  DVE    the two per-row Hadamards (q*k over [t,h,d], attn*v) and the
         LN apply; softmax ACT/PE ops are split into two independent
         half-tile chains so the in-order engines ping-pong without idling.
  DMA    attn [96,N] -> [128,N] head-dim replication and rstd [1,N] -> [32,N]
         broadcast via SBUF->SBUF partition-replicating APs on the GPSIMD
         (SWDGE) queue, keeping the sync queue free for input prefetch.
Pass A is emitted software-pipelined: loads(i+2) | proj/scores/softmax(i+1) |
ctx/LN(i).  Pass B (separate loop behind a scheduler fence, so the GELU act
table loads once): y@(lnw*Wd1) + lnb@Wd1 -> gelu -> @Wd2 -> gelu -> residual.
"""


import math

import numpy as np

B, DQ, DKV, T, A, H, O = 131072, 13, 32, 24, 32, 4, 32
HD = A // H
LN_EPS = 1e-5
NCORES = 8
BP = B // NCORES          # rows per core
N = 512                   # rows per tile (free dim)
NT = BP // N              # tiles per core (32)
G = 6                     # token groups of 4 per tile

_CACHE = {}


def _bf16():
    import ml_dtypes
    return ml_dtypes.bfloat16


def _act_table_patch():
    """Make bacc's act-table chooser resolve Exp and Ln to the combined
    natural_log_exp_and_others table (both funcs in one table -> one load per
    pass instead of four per tile). Only the chooser's view is altered; the
    emitted act_func_set_id still indexes the real act_info.json list, so the
    hardware loads a genuine table and numerics are unchanged."""
    import contextlib

    @contextlib.contextmanager
    def ctx():
        import concourse.bacc as bacc
        from concourse import mybir
        orig = bacc.get_activation_tables

        def patched(arch):
            t = dict(orig(arch))
            exp_f = mybir.ActivationFunctionType.Exp
            ln_f = mybir.ActivationFunctionType.Ln
            out = {}
            for name, funcs in t.items():
                if name == "natural_log_exp_and_others":
                    out[name] = funcs
                else:
                    out[name] = funcs - {exp_f, ln_f}
            return out

        bacc.get_activation_tables = patched
        try:
            yield
        finally:
            bacc.get_activation_tables = orig

    return ctx()


def _build(nt=NT, reps=1):
    import contextlib

    import concourse.bacc as bacc
    import concourse.bass as bass
    import concourse.tile as tile
    from concourse import mybir

    f32 = mybir.dt.float32
    bf16 = mybir.dt.bfloat16
    AF = mybir.ActivationFunctionType
    OP = mybir.AluOpType

    rows = nt * N
    nc = bacc.Bacc()

    qT_d = nc.dram_tensor("qT_in", [128, rows], bf16, kind="ExternalInput")
    kvT_d = nc.dram_tensor("kvT_in", [nt * 128, G * N], bf16, kind="ExternalInput")
    wkbd_d = nc.dram_tensor("wkbd", [128, 128], bf16, kind="ExternalInput")
    wvbd_d = nc.dram_tensor("wvbd", [128, 128], bf16, kind="ExternalInput")
    ssel_d = nc.dram_tensor("ssel", [128, 64], bf16, kind="ExternalInput")
    onesh_d = nc.dram_tensor("onesh", [96, 4], bf16, kind="ExternalInput")
    negexp_d = nc.dram_tensor("negexp", [4, 96], f32, kind="ExternalInput")
    wor_d = nc.dram_tensor("wor", [128, 33], bf16, kind="ExternalInput")
    esqw_d = nc.dram_tensor("esqw", [32, 1], bf16, kind="ExternalInput")
    ones32_d = nc.dram_tensor("ones32", [1, 32], bf16, kind="ExternalInput")
    wd1f_d = nc.dram_tensor("wd1f", [32, 32], bf16, kind="ExternalInput")
    b1row_d = nc.dram_tensor("b1row", [1, 32], bf16, kind="ExternalInput")
    wd2_d = nc.dram_tensor("wd2", [32, 32], bf16, kind="ExternalInput")
    lnw_d = nc.dram_tensor("lnwc", [32, 1], f32, kind="ExternalInput")
    lnb_d = nc.dram_tensor("lnbc", [32, 1], f32, kind="ExternalInput")
    out_d = nc.dram_tensor("outT", [O, rows], f32, kind="ExternalOutput")

    with tile.TileContext(nc) as tc:
        with (
            tc.tile_pool(name="consts", bufs=1) as consts,
            tc.tile_pool(name="kvin", bufs=4) as kvin,
            tc.tile_pool(name="sbig", bufs=2) as sbig,
            tc.tile_pool(name="ssm", bufs=2) as ssm,
            tc.tile_pool(name="yall", bufs=1) as yallp,
            tc.tile_pool(name="mlps", bufs=2) as mlps,
            tc.tile_pool(name="kvps", bufs=2, space="PSUM") as kvps,
            tc.tile_pool(name="sps", bufs=2, space="PSUM") as sps,
            tc.tile_pool(name="ops_", bufs=3, space="PSUM") as ops_,
            tc.tile_pool(name="dvps", bufs=1, space="PSUM") as dvps,
        ):
            wkbd = consts.tile([128, 128], bf16)
            wvbd = consts.tile([128, 128], bf16)
            ssel = consts.tile([128, 64], bf16)
            onesh = consts.tile([96, 4], bf16)
            negexp = consts.tile([4, 96], f32)
            wor = consts.tile([128, 33], bf16)
            esqw = consts.tile([32, 1], bf16)
            ones32 = consts.tile([1, 32], bf16)
            wd1f = consts.tile([32, 32], bf16)
            b1row = consts.tile([1, 32], bf16)
            wd2 = consts.tile([32, 32], bf16)
            lnw = consts.tile([32, 1], f32)
            lnb = consts.tile([32, 1], f32)
            onesrow = consts.tile([1, N], bf16)
            eps_sb = consts.tile([1, 1], f32)
            y_all = consts.tile([32, nt, N], bf16)

            for dst, src in [(wkbd, wkbd_d), (wvbd, wvbd_d),
                             (ssel, ssel_d), (onesh, onesh_d),
                             (negexp, negexp_d), (wor, wor_d), (esqw, esqw_d),
                             (ones32, ones32_d), (wd1f, wd1f_d),
                             (b1row, b1row_d), (wd2, wd2_d), (lnw, lnw_d),
                             (lnb, lnb_d)]:
                nc.sync.dma_start(out=dst, in_=src[tuple(slice(None) for _ in src.shape)])
            nc.vector.memset(onesrow, 1.0)
            nc.vector.memset(eps_sb, LN_EPS)

            loop_cm = tc.For_i(0, reps, 1) if reps > 1 else contextlib.nullcontext()
            with loop_cm:
                # ---- pass A, software-pipelined emission:
                #   L(i+2) loads | A1(i+1) proj+scores+softmax | A2(i) ctx+LN
                kv_t, q_t, attnrep_t = {}, {}, {}

                def stage_L(i):
                    kv_sb = kvin.tile([128, G, N], bf16, tag="kv", name=f"kv_{i}")
                    nc.sync.dma_start(out=kv_sb,
                                      in_=kvT_d[i * 128:(i + 1) * 128, :])
                    qrep = sbig.tile([128, N], bf16, tag="qrep", bufs=3,
                                     name=f"qrep_{i}")
                    nc.sync.dma_start(out=qrep, in_=qT_d[:, i * N:(i + 1) * N])
                    kv_t[i], q_t[i] = kv_sb, qrep

                def stage_A1(i):
                    kv_sb, qrep = kv_t[i], q_t[i]
                    prod1 = sbig.tile([128, G, N], bf16, tag="prod", bufs=4,
                                      name=f"prod1_{i}")
                    s_ps = sps.tile([128, N], f32, tag="scores", name=f"s_{i}")
                    for j in range(G):
                        k_ps = kvps.tile([128, N], f32, tag="kvp", name=f"k_{i}_{j}")
                        nc.tensor.matmul(k_ps, lhsT=wkbd, rhs=kv_sb[:, j, :])
                        if j < 2:
                            kc = sbig.tile([128, N], bf16, tag="kc", bufs=3,
                                           name=f"kc_{i}_{j}")
                            nc.scalar.copy(kc, k_ps)
                            nc.vector.tensor_mul(prod1[:, j, :], kc, qrep)
                        else:
                            nc.vector.tensor_mul(prod1[:, j, :], k_ps, qrep)
                        p = j // 2
                        nc.tensor.matmul(
                            s_ps[32 * p:32 * p + 32, :],
                            lhsT=ssel[:, 32 * (j % 2):32 * (j % 2) + 32],
                            rhs=prod1[:, j, :],
                            start=(j % 2 == 0), stop=(j % 2 == 1))
                    # softmax as two independent half-tile (N/2) chains,
                    # interleaved so ACT and PE ping-pong without idling
                    Nh = N // 2
                    exps = sbig.tile([96, N], bf16, tag="exps", name=f"exps_{i}")
                    attn = sbig.tile([96, N], bf16, tag="attn", name=f"attn_{i}")
                    lnden = ssm.tile([4, N], f32, tag="lnden", name=f"lnden_{i}")
                    hs = [slice(h * Nh, (h + 1) * Nh) for h in range(2)]
                    for h in hs:
                        nc.scalar.activation(exps[:, h], s_ps[0:96, h], AF.Exp)
                    for h in hs:
                        nc.tensor.matmul(s_ps[96:100, h], lhsT=onesh,
                                         rhs=exps[:, h],
                                         skip_group_check=True,
                                         tile_position=(0, 96))
                    for h in hs:
                        nc.scalar.activation(lnden[:, h], s_ps[96:100, h], AF.Ln)
                    for h in hs:
                        nc.tensor.matmul(s_ps[0:96, h], lhsT=negexp,
                                         rhs=lnden[:, h],
                                         start=False, stop=True,
                                         skip_group_check=True)
                    for h in hs:
                        nc.scalar.activation(attn[:, h], s_ps[0:96, h], AF.Exp)
                    attnrep = sbig.tile([128, G, N], bf16, tag="attnrep",
                                        bufs=3, name=f"attnrep_{i}")
                    for j in range(G):
                        sl = attn[16 * j:16 * j + 16, :]
                        rsrc = bass.AP(tensor=sl.tensor, offset=sl.offset,
                                       ap=[list(sl.ap[0]), [0, HD], [1, N]])
                        nc.gpsimd.dma_start(out=attnrep[:, j, :], in_=rsrc)
                    attnrep_t[i] = attnrep

                def stage_A2(i):
                    kv_sb, attnrep = kv_t.pop(i), attnrep_t.pop(i)
                    q_t.pop(i)
                    prod2 = sbig.tile([128, G, N], bf16, tag="prod", bufs=4,
                                      name=f"prod2_{i}")
                    o_ps = ops_.tile([128, N], f32, tag="out1", name=f"o_{i}")
                    for j in range(G):
                        v_ps = kvps.tile([128, N], f32, tag="kvp", name=f"v_{i}_{j}")
                        nc.tensor.matmul(v_ps, lhsT=wvbd, rhs=kv_sb[:, j, :])
                        if j < 2:
                            vc = sbig.tile([128, N], bf16, tag="kc", bufs=3,
                                           name=f"vc_{i}_{j}")
                            nc.scalar.copy(vc, v_ps)
                            nc.vector.tensor_mul(prod2[:, j, :], vc,
                                                 attnrep[:, j, :])
                        else:
                            nc.vector.tensor_mul(prod2[:, j, :], v_ps,
                                                 attnrep[:, j, :])
                        nc.tensor.matmul(o_ps[0:33, :], lhsT=wor,
                                         rhs=prod2[:, j, :],
                                         start=(j == 0), stop=(j == G - 1))
                    negmu = ssm.tile([1, N], bf16, tag="negmu", name=f"nmu_{i}")
                    nc.scalar.copy(negmu, o_ps[32:33, :])
                    nc.tensor.matmul(o_ps[0:32, :], lhsT=ones32, rhs=negmu,
                                     start=False, stop=True,
                                     skip_group_check=True)
                    sq = ssm.tile([32, N], bf16, tag="sq", name=f"sq_{i}")
                    nc.scalar.activation(sq, o_ps[0:32, :], AF.Square)
                    var_ps = dvps.tile([1, N], f32, tag="var", name=f"var_{i}")
                    nc.tensor.matmul(var_ps, lhsT=esqw, rhs=sq)
                    lnv = ssm.tile([1, N], f32, tag="lnv", name=f"lnv_{i}")
                    nc.scalar.activation(lnv, var_ps, AF.Ln, bias=eps_sb)
                    rstd = ssm.tile([1, N], bf16, tag="rstd", name=f"rstd_{i}")
                    nc.scalar.activation(rstd, lnv, AF.Exp, scale=-0.5)
                    rrep = ssm.tile([32, N], bf16, tag="rrep", name=f"rrep_{i}")
                    rsrc = bass.AP(tensor=rstd.tensor, offset=rstd.offset,
                                   ap=[list(rstd.ap[0]), [0, 32], [1, N]])
                    nc.gpsimd.dma_start(out=rrep, in_=rsrc)
                    nc.vector.tensor_mul(y_all[:, i, :], o_ps[0:32, :], rrep)

                stage_L(0)
                stage_L(1)
                stage_A1(0)
                for i in range(nt):
                    if i + 2 < nt:
                        stage_L(i + 2)
                    if i + 1 < nt:
                        stage_A1(i + 1)
                    stage_A2(i)

                # ---------------- pass B: gelu MLP + residual ----------------
                # scheduler fence: keeps all pass-B Gelu after pass-A exp/ln
                # in the ACT stream (2 act-table loads total, not per-tile)
                tc.no_sync_barrier()
                for i in range(nt):
                    n0 = i * N
                    h1_ps = kvps.tile([32, N], f32, tag="kvp", padded_shape=[128, N])
                    nc.tensor.matmul(h1_ps, lhsT=wd1f, rhs=y_all[:, i, :],
                                     start=True, stop=False)
                    nc.tensor.matmul(h1_ps, lhsT=b1row, rhs=onesrow,
                                     start=False, stop=True)
                    h1 = mlps.tile([32, N], bf16, tag="h1")
                    nc.scalar.activation(h1, h1_ps, AF.Gelu)
                    h2_ps = kvps.tile([32, N], f32, tag="kvp", padded_shape=[128, N])
                    nc.tensor.matmul(h2_ps, lhsT=wd2, rhs=h1)
                    h2 = mlps.tile([32, N], bf16, tag="h2")
                    nc.scalar.activation(h2, h2_ps, AF.Gelu)
                    t1 = mlps.tile([32, N], bf16, tag="t1")
                    nc.vector.scalar_tensor_tensor(t1, in0=y_all[:, i, :],
                                                   scalar=lnw, in1=h2,
                                                   op0=OP.mult, op1=OP.add)
                    fin = mlps.tile([32, N], f32, tag="fin")
                    nc.vector.tensor_scalar(fin, in0=t1, scalar1=lnb,
                                            scalar2=None, op0=OP.add)
                    nc.gpsimd.dma_start(out=out_d[:, n0:n0 + N], in_=fin)

    with _act_table_patch():
        nc.compile()
    return nc


def _prep_weights(Wq, Wk, Wv, Wo, ln_w, ln_b, Wd1, Wd2):
    bf = _bf16()
    Wq = np.asarray(Wq, np.float32)
    Wk = np.asarray(Wk, np.float32)
    Wv = np.asarray(Wv, np.float32)
    Wo = np.asarray(Wo, np.float32)
    Wd1 = np.asarray(Wd1, np.float32)
    Wd2 = np.asarray(Wd2, np.float32)
    ln_w = np.asarray(ln_w, np.float32)
    ln_b = np.asarray(ln_b, np.float32)
    scale = 1.0 / math.sqrt(HD)

    wkbd = np.zeros((128, 128), np.float32)
    wvbd = np.zeros((128, 128), np.float32)
    for tl in range(4):
        wkbd[32 * tl:32 * tl + 32, 32 * tl:32 * tl + 32] = Wk
        wvbd[32 * tl:32 * tl + 32, 32 * tl:32 * tl + 32] = Wv

    # ssel[:, 0:32] for even groups (slots 0-15), [:, 32:64] for odd (16-31)
    ssel = np.zeros((128, 64), np.float32)
    for par in range(2):
        for tl in range(4):
            for h in range(H):
                m = 16 * par + 4 * tl + h
                for d in range(HD):
                    ssel[32 * tl + 8 * h + d, 32 * par + m] = 1.0

    onesh = np.zeros((96, 4), np.float32)
    negexp = np.zeros((4, 96), np.float32)
    for t in range(T):
        for h in range(H):
            onesh[4 * t + h, h] = 1.0
            negexp[h, 4 * t + h] = -1.0

    wor = np.zeros((128, 33), np.float32)
    for tl in range(4):
        wor[32 * tl:32 * tl + 32, 0:32] = Wo
        wor[32 * tl:32 * tl + 32, 32] = -Wo.sum(axis=1) / O

    esqw = np.full((32, 1), 1.0 / O, np.float32)
    ones32 = np.ones((1, 32), np.float32)
    wd1f = ln_w[:, None] * Wd1
    b1row = (ln_b @ Wd1)[None, :]

    return {
        "wkbd": wkbd.astype(bf),
        "wvbd": wvbd.astype(bf), "ssel": ssel.astype(bf),
        "onesh": onesh.astype(bf), "negexp": negexp,
        "wor": wor.astype(bf), "esqw": esqw.astype(bf),
        "ones32": ones32.astype(bf),
        "wd1f": wd1f.astype(bf),
        "b1row": b1row.astype(bf), "wd2": Wd2.astype(bf),
        "lnwc": np.ascontiguousarray(ln_w[:, None]),
        "lnbc": np.ascontiguousarray(ln_b[:, None]),
    }


def _prep_inputs(query, kv, Wq):
    bf = _bf16()
    scale = 1.0 / math.sqrt(HD)
    Wq = np.asarray(Wq, np.float32)
    qproj = (np.asarray(query, np.float32) @ (Wq * scale)).astype(bf)  # [B,32]
    qrep = np.empty((128, B), bf)
    for tl in range(4):
        qrep[32 * tl:32 * tl + 32] = qproj.T
    # kv: [B, 768] -> per-core [768, BP] -> tiles [NT*128, 6*N] so each
    # SBUF partition reads one contiguous 6 KB strip per tile
    kvT = np.asarray(kv, np.float32).reshape(B, T * DKV).T.astype(bf)
    return qrep, kvT


def _retile_kv(kvT_core, nt=NT):
    # [768, rows] -> [nt*128, G*N]; row 128g+p col 512i+n -> [i*128+p, (g, n)]
    a = kvT_core.reshape(G, 128, nt, N).transpose(2, 1, 0, 3)
    return np.ascontiguousarray(a).reshape(nt * 128, G * N)


def kernel(query, kv, Wq, Wk, Wv, Wo, ln_w, ln_b, Wd1, Wd2):
    from concourse.bass_utils import run_bass_kernel_spmd

    if "nc" not in _CACHE:
        _CACHE["nc"] = _build()
    nc = _CACHE["nc"]

    w = _prep_weights(Wq, Wk, Wv, Wo, ln_w, ln_b, Wd1, Wd2)
    qrep, kvT = _prep_inputs(query, kv, Wq)

    in_maps = []
    for c in range(NCORES):
        m = dict(w)
        m["qT_in"] = np.ascontiguousarray(qrep[:, c * BP:(c + 1) * BP])
        m["kvT_in"] = _retile_kv(kvT[:, c * BP:(c + 1) * BP])
        in_maps.append(m)

    res = run_bass_kernel_spmd(nc, in_maps, core_ids=list(range(NCORES)),
                               trace=False)
    _CACHE["last_results"] = res
    out = np.concatenate([r["outT"].T for r in res.results], axis=0)
    return np.ascontiguousarray(out)

